# revision 35
# baseline (speedup 1.0000x reference)
"""Trainium2 Bass kernel for nn_MixedSparseSingleLayer (dense transformer layer
with LoRA adapters): RMSNorm -> QKV(+LoRA) -> RoPE -> causal attention ->
O-proj(+LoRA) -> residual -> RMSNorm -> MLP silu(up)+down (+LoRA) -> residual.

Sharding (8 NeuronCores): 2-way data parallel over batch x 4-way tensor
parallel (Megatron). Within a 4-core batch group:
  - norm1 is replicated (cheap), QKV is column-sharded so each core owns 4
    attention heads end-to-end (RoPE + causal softmax + PV).
  - attention outputs are exchanged with TWO AllToAlls (after heads 0-1 and
    2-3) so the exchange and the row-parallel O-proj overlap attention
    compute; each core then owns a 512-row slice for O-proj + MLP.
LoRA (rank 16), biases and RMSNorm weights are folded on the host (exact
algebraic rewrites). Weights, stored activations and the exchange payload
are bf16 (fp32 accumulate in PSUM); residuals/attention probabilities stay
fp32. V is produced directly in natural [rows, hd] layout (x-block
stationary, w_v moving) so PV needs no PE transposes.
"""

import numpy as np
import ml_dtypes

import concourse.bass as bass
import concourse.mybir as mybir
import concourse.tile as tile
from concourse import bacc
from concourse.bass_utils import run_bass_kernel_spmd

f32 = mybir.dt.float32
f32r = mybir.dt.float32r
bf16 = mybir.dt.bfloat16

B, S, D, H, HD, F, R = 2, 2048, 2048, 16, 128, 8192, 16
P = 128
KD = D // P            # 16 d_model tiles
MQK = 8                # q|k output tiles of the qkv shard (v handled apart)
NH = 4                 # heads per core
QB = S // P            # 16 s blocks
FT = F // P            # 64
ROWS = 512             # rows owned per core (S / 4)
SCALE = 1.0 / float(np.sqrt(HD))
EPS = 1e-10

N_CORES = 8
GROUPS = [[0, 1, 2, 3, 4, 5, 6, 7]]
RH2 = ROWS // 2        # 256: rows owned per core per batch
NSPLIT = 4             # a2a splits (heads per split = NH // NSPLIT)
HSP = NH // NSPLIT     # 2 heads per split


def build_program(single_core=False):
    nc = bacc.Bacc(
        "TRN2",
        target_bir_lowering=False,
        debug=False,
        num_devices=1 if single_core else N_CORES,
    )

    # ---- I/O ----
    xbT_in = nc.dram_tensor("xbT", [D, S], bf16, kind="ExternalInput").ap()
    xrT_in = nc.dram_tensor("xrT", [D, ROWS], f32, kind="ExternalInput").ap()
    wqk_in = nc.dram_tensor("wqk", [MQK, P, KD, P], bf16, kind="ExternalInput").ap()
    bqk_in = nc.dram_tensor("bqk", [P, MQK], f32, kind="ExternalInput").ap()
    wv_in = nc.dram_tensor("wv", [P, KD, NH * P], bf16, kind="ExternalInput").ap()
    bv_in = nc.dram_tensor("bv", [P, NH * P], f32, kind="ExternalInput").ap()
    wo_in = nc.dram_tensor("wo", [KD, P, KD, P], bf16, kind="ExternalInput").ap()
    wup_in = nc.dram_tensor("wup", [FT, P, KD, P], bf16, kind="ExternalInput").ap()
    bup_in = nc.dram_tensor("bup", [P, FT], f32, kind="ExternalInput").ap()
    wdn_in = nc.dram_tensor("wdn", [KD, P, FT, P], bf16, kind="ExternalInput").ap()
    bdn_in = nc.dram_tensor("bdn", [P, KD], f32, kind="ExternalInput").ap()
    cosT_in = nc.dram_tensor("cosT", [P, S], bf16, kind="ExternalInput").ap()
    sinTs_in = nc.dram_tensor("sinTs", [P, S], bf16, kind="ExternalInput").ap()
    rotmT_in = nc.dram_tensor("rotmT", [P, P], bf16, kind="ExternalInput").ap()
    ones_in = nc.dram_tensor("ones", [P, P], f32r, kind="ExternalInput").ap()
    mask_in = nc.dram_tensor("mask", [P, 512], f32, kind="ExternalInput").ap()
    outT = nc.dram_tensor("outT", [D, ROWS], f32, kind="ExternalOutput").ap()

    with tile.TileContext(nc) as tc:
        _emit(tc, nc, xbT_in, xrT_in, wqk_in, bqk_in, wv_in, bv_in, wo_in,
              wup_in, bup_in, wdn_in, bdn_in, cosT_in, sinTs_in, rotmT_in,
              ones_in, mask_in, outT, single_core)

    nc.compile()
    return nc


def _emit(tc, nc, xbT_in, xrT_in, wqk_in, bqk_in, wv_in, bv_in, wo_in,
          wup_in, bup_in, wdn_in, bdn_in, cosT_in, sinTs_in, rotmT_in,
          ones_in, mask_in, outT, single_core=False):
    from contextlib import ExitStack

    top = ExitStack()
    with top:
        consts = top.enter_context(tc.tile_pool(name="consts", bufs=1))
        # only what the first chunk's stats/matmuls need is DMA'd up front;
        # bulky constants are emitted at first use so they don't delay the
        # initial x chunk in the DMA queues.
        ones = consts.tile([P, P], f32r, tag="ones")
        nc.sync.dma_start(ones[:], ones_in)
        ones_bf = consts.tile([P, 1], bf16, tag="ones_bf")
        nc.vector.memset(ones_bf[:], 1.0)
        wmask_sb = consts.tile([P, 512], f32, tag="mask")
        cosT = consts.tile([P, S], bf16, tag="cosT")
        sinTs = consts.tile([P, S], bf16, tag="sinTs")
        rotmT = consts.tile([P, P], bf16, tag="rotmT")
        nc.sync.dma_start(rotmT[:], rotmT_in)
        bqk_sb = consts.tile([P, MQK], f32, tag="bqk")
        nc.sync.dma_start(bqk_sb[:], bqk_in)
        bv_sb = consts.tile([P, NH * P], f32, tag="bv")
        bup_sb = consts.tile([P, FT], f32, tag="bup")
        bdn_sb = consts.tile([P, KD], f32, tag="bdn")
        eps_sb = consts.tile([P, 1], f32, tag="eps")
        nc.vector.memset(eps_sb[:], EPS)
        scr_sb = consts.tile([P, 1], f32, tag="scr")
        nc.vector.memset(scr_sb[:], 0.0)
        sqr2 = consts.tile([1, ROWS], f32, tag="sqr2")
        rr2 = consts.tile([1, ROWS], f32, tag="rr2")

        # DRAM staging for the two AllToAll exchanges (bf16 payload).
        # Split s carries heads [2s, 2s+1]; chunk j holds those heads' dims
        # (256) x core j's 256 owned rows of THIS core's batch.
        dram = top.enter_context(tc.tile_pool(name="a2a", bufs=1, space="DRAM"))
        a2a_in = [dram.tile([N_CORES, HSP * P, RH2], bf16, tag=f"a2a_in{s}",
                            name=f"a2a_in{s}") for s in range(NSPLIT)]
        a2a_out = [dram.tile([N_CORES, HSP * P, RH2], bf16, tag=f"a2a_out{s}",
                             name=f"a2a_out{s}") for s in range(NSPLIT)]

        # x1T (residual accumulator) and the norm2-stats bank outlive the
        # attention pools, so allocate them first (pool scopes are LIFO).
        x1_stack = ExitStack()
        x1p = x1_stack.enter_context(tc.tile_pool(name="x1T", bufs=1))
        x1T = x1p.tile([P, KD * ROWS], f32, tag="x1T")
        # ============ Phase A: norm1 + QK (transposed) + V (natural) ========
        qkv_stack = ExitStack()
        qkvp = qkv_stack.enter_context(tc.tile_pool(name="qkT", bufs=1))
        qkT = qkvp.tile([P, MQK * S], bf16, tag="qkT")
        vnatp = qkv_stack.enter_context(tc.tile_pool(name="vnat", bufs=1))
        # natural-layout V for all 4 heads: block kt is rows [128k,128k+128),
        # head h at columns kt*512 + h*128
        vnat = vnatp.tile([P, QB * NH * P], bf16, tag="vnat")
        wvp = qkv_stack.enter_context(tc.tile_pool(name="wv", bufs=1))
        wv_sb = wvp.tile([P, KD * NH * P], bf16, tag="wv")

        with tc.tile_pool(name="phA_sb", bufs=2) as pa, \
             tc.tile_pool(name="phA_sq", bufs=16) as sqp, \
             tc.tile_pool(name="phA_w", bufs=4) as wp, \
             tc.tile_pool(name="phA_ps", bufs=2, space="PSUM") as pps, \
             tc.tile_pool(name="phA_vps", bufs=2, space="PSUM") as vps, \
             tc.tile_pool(name="phA_st", bufs=2, space="PSUM") as stps, \
             tc.tile_pool(name="phA_rt", bufs=3) as rtp, \
             tc.tile_pool(name="phA_rps", bufs=1, space="PSUM") as rops, \
             tc.tile_pool(name="phA_r", bufs=2) as rp:
            NRH = 512  # rows per chunk
            NCH = S // NRH
            xn1_t = {}

            def load_chunk(c, kd0=0, kd1=KD):
                if kd0 == 0:
                    xn1_t[c] = pa.tile([P, KD * NRH], bf16, tag="xn1",
                                       name=f"xn1_{c}")
                t = xn1_t[c]
                for kd in range(kd0, kd1):
                    nc.sync.dma_start(
                        t[:, kd * NRH:(kd + 1) * NRH],
                        xbT_in[kd * P:(kd + 1) * P, c * NRH:(c + 1) * NRH])

            load_chunk(0)

            def norm_chunk(c):
                # row stats: ssq[r] = sum_d x[d,r]^2 (PE ones-matmul trick),
                # then normalize xn1 in place
                xn1 = xn1_t[c]
                st = stps.tile([P, NRH], f32, tag="st", name=f"st_{c}")
                ssq = st[0:1, :]
                for kd in range(KD):
                    sq = sqp.tile([P, NRH], f32, tag="sq")
                    nc.scalar.activation(sq[:].bitcast(f32r),
                                         xn1[:, kd * NRH:(kd + 1) * NRH],
                                         mybir.ActivationFunctionType.Square)
                    nc.tensor.matmul(
                        ssq, ones[:, 0:1], sq[:].bitcast(f32r),
                        start=(kd == 0), stop=(kd == KD - 1))
                sqr = rp.tile([1, NRH], f32, tag="sqr")
                nc.scalar.activation(sqr[:], ssq,
                                     mybir.ActivationFunctionType.Sqrt,
                                     bias=eps_sb[0:1, :], scale=1.0 / D)
                rr = rp.tile([1, NRH], f32, tag="rr")
                with nc.allow_low_precision(reason="f32r rounding for PE broadcast"):
                    nc.vector.reciprocal(rr[:].bitcast(f32r), sqr[:])
                rb = st
                nc.tensor.matmul(rb[:], ones[0:1, :],
                                 rr[:].bitcast(f32r), start=True, stop=True)
                if c == NCH - 1:
                    # preload the exp ACT table; Identity/Square stay valid
                    nc.scalar.activation(scr_sb[:], scr_sb[:],
                                         mybir.ActivationFunctionType.Exp)
                for kd in range(KD):
                    nc.vector.tensor_mul(xn1[:, kd * NRH:(kd + 1) * NRH],
                                         xn1[:, kd * NRH:(kd + 1) * NRH],
                                         rb[:])

            norm_chunk(0)
            for rh in range(NCH):
                xn1 = xn1_t.pop(rh)
                def v_block(sb):
                    kt = rh * (NRH // P) + sb
                    vacc = vps.tile([P, NH * P], f32, tag="vacc")
                    for kd in range(KD):
                        nc.tensor.matmul(
                            vacc[:],
                            xn1[:, kd * NRH + sb * P: kd * NRH + (sb + 1) * P],
                            wv_sb[:, kd * NH * P:(kd + 1) * NH * P],
                            start=(kd == 0), stop=(kd == KD - 1))
                    nc.vector.tensor_add(
                        vnat[:, kt * NH * P:(kt + 1) * NH * P], vacc[:], bv_sb[:])

                # QK matmuls: head-major m order (q_h = mt h, k_h = mt 4+h)
                # followed by in-place RoPE on this chunk's rows; V-blocks
                # interleave so the DVE work stays spread out.
                for mj, mt in enumerate((0, 4, 1, 5, 2, 6, 3, 7)):
                    wsb = wp.tile([P, KD * P], bf16, tag="wqk")
                    nc.sync.dma_start(
                        wsb[:], wqk_in[mt].rearrange("p k m -> p (k m)"))
                    if rh == 0 and mt == 0:
                        nc.sync.dma_start(cosT[:], cosT_in)
                        nc.sync.dma_start(sinTs[:], sinTs_in)
                    if rh + 1 < NCH:
                        if mt == 4:
                            load_chunk(rh + 1, 0, 5)
                        elif mt == 1:
                            load_chunk(rh + 1, 5, 10)
                        elif mt == 5:
                            load_chunk(rh + 1, 10, KD)
                    acc = pps.tile([P, NRH], f32, tag="qkacc")
                    for kd in range(KD):
                        nc.tensor.matmul(
                            acc[:],
                            wsb[:, kd * P:(kd + 1) * P],
                            xn1[:, kd * NRH:(kd + 1) * NRH],
                            start=(kd == 0), stop=(kd == KD - 1))
                    qk_sl = qkT[:, mt * S + rh * NRH: mt * S + rh * NRH + NRH]
                    # bias-add + cast on DVE (keeps ACT free for the next
                    # chunk's stats squares)
                    nc.vector.tensor_scalar_add(qk_sl, acc[:],
                                                bqk_sb[:, mt:mt + 1])
                    cs_sl = slice(rh * NRH, (rh + 1) * NRH)
                    rt = rops.tile([P, NRH], f32, tag="ropt")
                    nc.tensor.matmul(rt[:], rotmT[:], qk_sl,
                                     start=True, stop=True)
                    rtmp = rtp.tile([P, NRH], bf16, tag="rtmp")
                    nc.vector.tensor_mul(rtmp[:], rt[:], sinTs[:, cs_sl])
                    nc.vector.tensor_mul(qk_sl, qk_sl, cosT[:, cs_sl])
                    nc.vector.tensor_add(qk_sl, qk_sl, rtmp[:])
                if rh == 0:
                    nc.sync.dma_start(bv_sb[:], bv_in)
                    for kd in range(KD):
                        nc.sync.dma_start(
                            wv_sb[:, kd * NH * P:(kd + 1) * NH * P],
                            wv_in[:, kd, :])
                if rh + 1 < NCH:
                    norm_chunk(rh + 1)
                for sb in range(NRH // P):
                    v_block(sb)

        # ====== Phase B + C: attention, split AllToAll, partial O-proj ======
        nc.sync.dma_start(wmask_sb[:], mask_in)
        # preload residual rows (+b_o) straight into x1T
        for kd in range(KD):
            nc.sync.dma_start(x1T[:, kd * ROWS:(kd + 1) * ROWS],
                              xrT_in[kd * P:(kd + 1) * P, :])

        with tc.tile_pool(name="prT", bufs=5) as prtp, \
             tc.tile_pool(name="lsum", bufs=4) as lp, \
             tc.tile_pool(name="rbc", bufs=2) as rbcp, \
             tc.tile_pool(name="ocp", bufs=2) as ocpp, \
             tc.tile_pool(name="oT", bufs=2) as otp, \
             tc.tile_pool(name="phC_om", bufs=2) as omp, \
             tc.tile_pool(name="phC_w", bufs=32) as wop, \
             tc.tile_pool(name="phC_sq", bufs=3) as sqp2, \
             tc.tile_pool(name="sc_ps", bufs=2, space="PSUM") as scps, \
             tc.tile_pool(name="ov_ps", bufs=1, space="PSUM") as ovps, \
             tc.tile_pool(name="phC_ps", bufs=2, space="PSUM") as cps, \
             tc.tile_pool(name="st_ps", bufs=1, space="PSUM") as stp2:

            def attention_head(h):
                rq = qkT[:, h * S:(h + 1) * S]
                rk = qkT[:, (NH + h) * S:(NH + h + 1) * S]
                oTh = otp.tile([P, S], bf16, tag="oTh")
                # q processed in 512-wide chunks; scores computed TRANSPOSED
                # (s.T[S_k, q]) so exp output is already in PV layout.
                # Software pipelining: each kt's lps/PV matmuls are deferred
                # two score-matmuls so the PE never waits on ACT exp, and
                # each qc's 1/l normalization is deferred into the next qc
                # so the PE never waits on the DVE reciprocal.
                pend = [None]

                def flush():
                    if pend[0] is None:
                        return
                    ocopy_p, lr_p, rinv_p, dst = pend[0]
                    pend[0] = None
                    nc.tensor.matmul(lr_p[:], ones[0:1, :],
                                     rinv_p[:].bitcast(f32r),
                                     start=True, stop=True)
                    rbs = rbcp.tile([P, 512], f32, tag="rbs")
                    nc.vector.tensor_copy(rbs[:], lr_p[:])
                    nc.vector.tensor_mul(dst, ocopy_p[:], rbs[:])

                for qc in range(S // 512):
                    opsum = ovps.tile([P, 512], f32, tag="opv")
                    lr = stp2.tile([P, 512], f32, tag="lr")
                    lps = lr[0:1, :]
                    nkt = 4 * qc + 4

                    def lps_pv(prT_sl, kt, q0, w, nkt=nkt, lr=lr,
                               opsum=opsum, h=h):
                        # partial-width accumulates: causal diagonal blocks
                        # only cover q columns >= their own k rows
                        nc.tensor.matmul(
                            lr[0:1, q0:512], ones_bf[:], prT_sl,
                            start=(kt == 0), stop=(kt == nkt - 1),
                            skip_group_check=True)
                        nc.tensor.matmul(
                            opsum[:, q0:512],
                            vnat[:, kt * NH * P + h * P: kt * NH * P + (h + 1) * P],
                            prT_sl,
                            start=(kt == 0), stop=(kt == nkt - 1),
                            skip_group_check=True)

                    todo = []
                    emitted = [0]

                    def drain_todo(upto):
                        while emitted[0] < upto:
                            lps_pv(*todo[emitted[0]])
                            emitted[0] += 1

                    pair = [None]
                    for kt in range(nkt):
                        lb = kt - 4 * qc
                        q0 = max(lb, 0) * P
                        w = 512 - q0
                        if lb < 0:
                            # full-width block: pack two per PSUM pair-tile,
                            # one exp call per pair (ACT dispatch is pricey)
                            if pair[0] is None:
                                pt = scps.tile([P, 1024], f32, tag="scc")
                                pair[0] = (pt, [])
                            pt, members = pair[0]
                            half = len(members)
                            nc.tensor.matmul(
                                pt[:, half * 512:(half + 1) * 512],
                                rk[:, kt * P:(kt + 1) * P],
                                rq[:, qc * 512:(qc + 1) * 512],
                                start=True, stop=True)
                            members.append(kt)
                            if kt == 0:
                                flush()
                            if len(members) == 2:
                                prT = prtp.tile([P, 1024], bf16, tag="prT")
                                nc.scalar.activation(
                                    prT[:], pt[:],
                                    mybir.ActivationFunctionType.Exp,
                                    scale=SCALE)
                                todo.append((prT[:, 0:512], members[0], 0, 512))
                                todo.append((prT[:, 512:1024], members[1], 0, 512))
                                pair[0] = None
                        else:
                            scc = scps.tile([P, 1024], f32, tag="scc")
                            nc.tensor.matmul(
                                scc[:, 0:w],
                                rk[:, kt * P:(kt + 1) * P],
                                rq[:, qc * 512 + q0:(qc + 1) * 512],
                                start=True, stop=True)
                            if kt == 0:
                                flush()
                            # triangular mask on the block-diagonal 128 cols
                            nc.vector.tensor_add(
                                scc[:, 0:P], scc[:, 0:P],
                                wmask_sb[:, 384:512])
                            prT = prtp.tile([P, 1024], bf16, tag="prT")
                            nc.scalar.activation(
                                prT[:, 0:w], scc[:, 0:w],
                                mybir.ActivationFunctionType.Exp, scale=SCALE)
                            todo.append((prT[:, 0:w], kt, q0, w))
                        drain_todo(len(todo) - 2)
                    drain_todo(nkt)
                    rinv = lp.tile([1, 512], f32, tag="rinv")
                    with nc.allow_low_precision(reason="f32r rounding for PE bcast"):
                        nc.vector.reciprocal(rinv[:].bitcast(f32r), lps)
                    # evacuate the PV sum to SBUF so the PSUM bank frees now
                    ocopy = ocpp.tile([P, 512], f32, tag="ocopy")
                    nc.vector.tensor_copy(ocopy[:], opsum[:])
                    pend[0] = (ocopy, lr, rinv,
                               oTh[:, qc * 512:(qc + 1) * 512])
                flush()
                # stage this head's output for its a2a split
                s, hh = divmod(h, HSP)
                for j in range(N_CORES):
                    nc.sync.dma_start(
                        a2a_in[s][j, hh * P:(hh + 1) * P, :],
                        oTh[:, j * RH2:(j + 1) * RH2])

            def exchange(s):
                if single_core:
                    # timing-only stand-in for the collective: move the same
                    # bytes DRAM->DRAM locally
                    nc.sync.dma_start(
                        a2a_out[s][:].rearrange("a r c -> (a r) c"),
                        a2a_in[s][:].rearrange("a r c -> (a r) c"))
                else:
                    nc.gpsimd.collective_compute(
                        "AllToAll", mybir.AluOpType.bypass,
                        replica_groups=GROUPS,
                        ins=[a2a_in[s][:].opt()],
                        outs=[a2a_out[s][:].opt()],
                    )

            NK = KD // NSPLIT
            wo_tiles = {}

            def load_wo(s):
                for mt in range(KD):
                    wsb = wop.tile([P, NK * P], bf16, tag="wo",
                                   name=f"wo_{s}_{mt}")
                    nc.sync.dma_start(
                        wsb[:], wo_in[mt][:, s * NK:(s + 1) * NK, :]
                        .rearrange("p k m -> p (k m)"))
                    wo_tiles[(s, mt)] = wsb

            def oproj_partial(s):
                # split s delivers kds {4g + 2s, 4g + 2s + 1 : g in 0..3}
                kds = [4 * g + HSP * s + i for g in range(4) for i in range(HSP)]
                om = omp.tile([P, len(kds) * ROWS], bf16, tag="om")
                for ci, kd in enumerate(kds):
                    g, i = kd // 4, kd % 4 - HSP * s
                    for b in range(B):
                        nc.sync.dma_start(
                            om[:, ci * ROWS + b * RH2: ci * ROWS + (b + 1) * RH2],
                            a2a_out[s][4 * b + g, i * P:(i + 1) * P, :])
                nk = len(kds)
                for mt in range(KD):
                    wsb = wo_tiles.pop((s, mt))
                    acc = cps.tile([P, ROWS], f32, tag="oacc")
                    for ci in range(nk):
                        nc.tensor.matmul(
                            acc[:], wsb[:, ci * P:(ci + 1) * P],
                            om[:, ci * ROWS:(ci + 1) * ROWS],
                            start=(ci == 0), stop=(ci == nk - 1))
                    nc.vector.tensor_add(x1T[:, mt * ROWS:(mt + 1) * ROWS],
                                         x1T[:, mt * ROWS:(mt + 1) * ROWS],
                                         acc[:])
                    if s == NSPLIT - 1:
                        # x1T[mt] final: fold norm2 stats in right away
                        if mt == 0:
                            st2_t[0] = stp2.tile([P, 512], f32, tag="lr",
                                                 name="st2")
                        sq = sqp2.tile([P, ROWS], f32, tag="sq2")
                        nc.scalar.activation(
                            sq[:].bitcast(f32r), x1T[:, mt * ROWS:(mt + 1) * ROWS],
                            mybir.ActivationFunctionType.Square)
                        nc.tensor.matmul(st2_t[0][0:1, :], ones[:, 0:1],
                                         sq[:].bitcast(f32r),
                                         start=(mt == 0), stop=(mt == KD - 1))

            # pipeline: each split's exchange flies while the next head's
            # attention keeps the PE busy; its O-proj partial lands after.
            for h in range(NH):
                attention_head(h)
                if (h + 1) % HSP == 0:
                    load_wo((h + 1) // HSP - 1)
                    exchange((h + 1) // HSP - 1)
                    if (h + 1) // HSP - 2 >= 0:
                        oproj_partial((h + 1) // HSP - 2)
            # preload the sqrt ACT table while the last O-proj runs
            nc.scalar.activation(scr_sb[:], scr_sb[:],
                                 mybir.ActivationFunctionType.Sqrt)
            st2_t = [None]
            oproj_partial(NSPLIT - 1)
            # norm2 scale while the stats bank is still allocated
            nc.scalar.activation(sqr2[:], st2_t[0][0:1, :],
                                 mybir.ActivationFunctionType.Sqrt,
                                 bias=eps_sb[0:1, :], scale=1.0 / D)
            with nc.allow_low_precision(reason="f32r rounding for PE broadcast"):
                nc.vector.reciprocal(rr2[:].bitcast(f32r), sqr2[:])
        qkv_stack.close()

        # ================= Phase D..F: norm2 + MLP ==========================
        mlp_stack = ExitStack()
        xn2p = mlp_stack.enter_context(tc.tile_pool(name="xn2", bufs=1))
        fnp = mlp_stack.enter_context(tc.tile_pool(name="fnT", bufs=1))
        xn2 = xn2p.tile([P, KD * ROWS], bf16, tag="xn2")
        fnT = fnp.tile([P, FT * ROWS], bf16, tag="fnT")

        nc.sync.dma_start(bup_sb[:], bup_in)
        nc.sync.dma_start(bdn_sb[:], bdn_in)
        with tc.tile_pool(name="phD_rb", bufs=1, space="PSUM") as rbps, \
             tc.tile_pool(name="phE_w", bufs=4) as wup_p, \
             tc.tile_pool(name="phF_w", bufs=2) as wdn_p, \
             tc.tile_pool(name="phF_out", bufs=2) as outp, \
             tc.tile_pool(name="phE_ps", bufs=4, space="PSUM") as eps_ps, \
             tc.tile_pool(name="phF_ps", bufs=3, space="PSUM") as fps:
            rb2 = rbps.tile([P, ROWS], f32, tag="rb2")
            nc.tensor.matmul(rb2[:], ones[0:1, :],
                             rr2[:].bitcast(f32r), start=True, stop=True)
            for kd in range(KD):
                nc.vector.tensor_mul(xn2[:, kd * ROWS:(kd + 1) * ROWS],
                                     x1T[:, kd * ROWS:(kd + 1) * ROWS], rb2[:])

            for mt in range(FT):
                wsb = wup_p.tile([P, KD * P], bf16, tag="wup")
                nc.sync.dma_start(wsb[:], wup_in[mt].rearrange("p k m -> p (k m)"))
                acc = eps_ps.tile([P, ROWS], f32, tag="upacc")
                for kd in range(KD):
                    nc.tensor.matmul(
                        acc[:], wsb[:, kd * P:(kd + 1) * P],
                        xn2[:, kd * ROWS:(kd + 1) * ROWS],
                        start=(kd == 0), stop=(kd == KD - 1))
                # fn = silu(up + b_up), cast to bf16
                nc.scalar.activation(fnT[:, mt * ROWS:(mt + 1) * ROWS], acc[:],
                                     mybir.ActivationFunctionType.Silu,
                                     bias=bup_sb[:, mt:mt + 1])

            for mt in range(KD):
                wsb = wdn_p.tile([P, FT * P], bf16, tag="wdn")
                nc.sync.dma_start(wsb[:], wdn_in[mt].rearrange("p k m -> p (k m)"))
                acc = fps.tile([P, ROWS], f32, tag="dnacc")
                for kd in range(FT):
                    nc.tensor.matmul(
                        acc[:], wsb[:, kd * P:(kd + 1) * P],
                        fnT[:, kd * ROWS:(kd + 1) * ROWS],
                        start=(kd == 0), stop=(kd == FT - 1))
                out_sb = outp.tile([P, ROWS], f32, tag="out_sb")
                for hh in range(2):
                    cs = slice(hh * (ROWS // 2), (hh + 1) * (ROWS // 2))
                    nc.vector.scalar_tensor_tensor(
                        out_sb[:, cs], acc[:, cs], bdn_sb[:, mt:mt + 1],
                        x1T[:, mt * ROWS + hh * (ROWS // 2):
                            mt * ROWS + (hh + 1) * (ROWS // 2)],
                        op0=mybir.AluOpType.add, op1=mybir.AluOpType.add)
                    nc.sync.dma_start(outT[mt * P:(mt + 1) * P, cs],
                                      out_sb[:, cs])
        mlp_stack.close()
        x1_stack.close()


def host_prepare(inputs):
    """Fold LoRA/norm-weights/biases and build the 8 per-core input maps."""
    gi = {k: np.asarray(v, dtype=np.float32) if np.asarray(v).dtype != np.float32
          else np.asarray(v) for k, v in inputs.items()}

    def fold(nm):
        return gi['w_' + nm] + gi['w_' + nm + '_lora_a'] @ gi['w_' + nm + '_lora_b']

    nw1 = gi['norm_weight_1'][:, None]
    nw2 = gi['norm_weight_2'][:, None]
    w_q = (nw1 * fold('q')).astype(np.float32)
    w_k = (nw1 * fold('k')).astype(np.float32)
    w_v = (nw1 * fold('v')).astype(np.float32)
    w_o = fold('o').astype(np.float32)
    w_up = (nw2 * fold('up')).astype(np.float32)
    w_dn = fold('down').astype(np.float32)

    # pre-tiled weight layouts [mt, p, kd, m]
    kd_order = [4 * g + HSP * sp + i
                for sp in range(NSPLIT) for g in range(4) for i in range(HSP)]
    wo_t = np.ascontiguousarray(
        w_o.reshape(KD, P, KD, P).transpose(2, 1, 0, 3)[:, :, kd_order]).astype(
            ml_dtypes.bfloat16)
    wup_t = np.ascontiguousarray(
        w_up.reshape(KD, P, FT, P).transpose(2, 1, 0, 3)).astype(ml_dtypes.bfloat16)
    wdn_t = np.ascontiguousarray(
        w_dn.reshape(FT, P, KD, P).transpose(2, 1, 0, 3)).astype(ml_dtypes.bfloat16)
    bup_t = np.ascontiguousarray(gi['b_up'].reshape(FT, P).T)
    bdn_t = np.ascontiguousarray(gi['b_down'].reshape(KD, P).T)

    cosT = np.ascontiguousarray(gi['cos'].T).astype(ml_dtypes.bfloat16)
    sinTs = np.ascontiguousarray(gi['sin'].T).astype(ml_dtypes.bfloat16)
    # rot(x).T = R @ x.T with R[d, d+64] = -1 (d<64), R[d, d-64] = +1;
    # matmul computes lhsT.T @ rhs, so pass R.T.
    Rm = np.zeros((P, P), dtype=np.float32)
    hh = HD // 2
    Rm[np.arange(hh), np.arange(hh) + hh] = -1.0
    Rm[np.arange(hh) + hh, np.arange(hh)] = 1.0
    rotmT = np.ascontiguousarray(Rm.T).astype(ml_dtypes.bfloat16)
    maskT = np.maximum(gi['attention_mask'][0, 0, :P, :P], -2000.0).T
    wmask = np.full((P, 512), -2000.0, dtype=np.float32)
    wmask[:, 384:512] = maskT
    mask128 = np.ascontiguousarray(wmask)

    x = gi['x']
    b_o = gi['b_o']
    in_maps = []
    for i in range(N_CORES):
        b, g = divmod(i, 4)
        hs = slice(512 * g, 512 * (g + 1))
        wqk = np.concatenate([w_q[:, hs], w_k[:, hs]], axis=1)
        wqk_t = np.ascontiguousarray(
            wqk.reshape(KD, P, MQK, P).transpose(2, 1, 0, 3)).astype(ml_dtypes.bfloat16)
        bqk = np.concatenate([gi['b_q'][hs], gi['b_k'][hs]])
        bqk_t = np.ascontiguousarray(bqk.reshape(MQK, P).T)
        # V natural-layout weights: [p, kd, m] so the kd-th moving slice is
        # w_v rows [128kd, 128kd+128) x this core's 512 head-dims
        wv_t = np.ascontiguousarray(
            w_v[:, hs].reshape(KD, P, NH * P).transpose(1, 0, 2)).astype(
                ml_dtypes.bfloat16)
        bv_t = np.broadcast_to(gi['b_v'][hs], (P, NH * P)).copy()
        xbT = np.ascontiguousarray(x[b].T).astype(ml_dtypes.bfloat16)
        # this core owns rows [256i, 256(i+1)) of BOTH batches
        xrows = np.concatenate(
            [x[0, RH2 * i:RH2 * (i + 1)], x[1, RH2 * i:RH2 * (i + 1)]], axis=0)
        xrT = np.ascontiguousarray(xrows.T + b_o[:, None])
        in_maps.append({
            "xbT": xbT, "xrT": xrT,
            "wqk": wqk_t, "bqk": bqk_t, "wv": wv_t, "bv": bv_t,
            "wo": wo_t, "wup": wup_t, "bup": bup_t,
            "wdn": wdn_t, "bdn": bdn_t,
            "cosT": cosT, "sinTs": sinTs, "rotmT": rotmT,
            "ones": np.ones((P, P), dtype=np.float32), "mask": mask128,
        })
    return in_maps


def assemble(results):
    out = np.empty((B, S, D), dtype=np.float32)
    for i in range(N_CORES):
        oT = results[i]["outT"]
        out[0, RH2 * i:RH2 * (i + 1), :] = oT[:, 0:RH2].T
        out[1, RH2 * i:RH2 * (i + 1), :] = oT[:, RH2:ROWS].T
    return out


_NC_CACHE = {}


def get_nc():
    if "nc" not in _NC_CACHE:
        _NC_CACHE["nc"] = build_program()
    return _NC_CACHE["nc"]


def kernel(**inputs):
    nc = get_nc()
    in_maps = host_prepare(inputs)
    res = run_bass_kernel_spmd(nc, in_maps, list(range(N_CORES)))
    return assemble(res.results)


# revision 60
# speedup vs baseline: 35625.6264x; 35625.6264x over previous
"""Trainium2 Bass kernel for nn_MixedSparseSingleLayer (dense transformer layer
with LoRA adapters): RMSNorm -> QKV(+LoRA) -> RoPE -> causal attention ->
O-proj(+LoRA) -> residual -> RMSNorm -> MLP silu(up)+down (+LoRA) -> residual.

Sharding (8 NeuronCores): 2-way data parallel over batch x 4-way tensor
parallel (Megatron). Within a 4-core batch group:
  - norm1 is replicated (cheap), QKV is column-sharded so each core owns 4
    attention heads end-to-end (RoPE + causal softmax + PV).
  - attention outputs are exchanged with TWO AllToAlls (after heads 0-1 and
    2-3) so the exchange and the row-parallel O-proj overlap attention
    compute; each core then owns a 512-row slice for O-proj + MLP.
LoRA (rank 16), biases and RMSNorm weights are folded on the host (exact
algebraic rewrites). Weights, stored activations and the exchange payload
are bf16 (fp32 accumulate in PSUM); residuals/attention probabilities stay
fp32. V is produced directly in natural [rows, hd] layout (x-block
stationary, w_v moving) so PV needs no PE transposes.
"""

import numpy as np
import ml_dtypes

import concourse.bass as bass
import concourse.mybir as mybir
import concourse.tile as tile
from concourse import bacc
from concourse.bass_utils import run_bass_kernel_spmd

f32 = mybir.dt.float32
f32r = mybir.dt.float32r
bf16 = mybir.dt.bfloat16

B, S, D, H, HD, F, R = 2, 2048, 2048, 16, 128, 8192, 16
P = 128
KD = D // P            # 16 d_model tiles
MQK = 8                # q|k output tiles of the qkv shard (v handled apart)
NH = 4                 # heads per core
QB = S // P            # 16 s blocks
FT = F // P            # 64
ROWS = 512             # rows owned per core (S / 4)
SCALE = 1.0 / float(np.sqrt(HD))
EPS = 1e-10

N_CORES = 8
GROUPS = [[0, 1, 2, 3, 4, 5, 6, 7]]
RH2 = ROWS // 2        # 256: rows owned per core per batch
NSPLIT = 4             # a2a splits (heads per split = NH // NSPLIT)
HSP = NH // NSPLIT     # 2 heads per split


def build_program(single_core=False):
    nc = bacc.Bacc(
        "TRN2",
        target_bir_lowering=False,
        debug=False,
        num_devices=1 if single_core else N_CORES,
    )

    # ---- I/O ----
    xbT_in = nc.dram_tensor("xbT", [D, S], bf16, kind="ExternalInput").ap()
    xrT_in = nc.dram_tensor("xrT", [D, ROWS], f32, kind="ExternalInput").ap()
    wqk_in = nc.dram_tensor("wqk", [MQK, P, KD, P], bf16, kind="ExternalInput").ap()
    bqk_in = nc.dram_tensor("bqk", [P, MQK], f32, kind="ExternalInput").ap()
    wv_in = nc.dram_tensor("wv", [P, KD, NH * P], bf16, kind="ExternalInput").ap()
    bv_in = nc.dram_tensor("bv", [P, NH * P], f32, kind="ExternalInput").ap()
    wo_in = nc.dram_tensor("wo", [KD, P, KD, P], bf16, kind="ExternalInput").ap()
    wup_in = nc.dram_tensor("wup", [FT, P, KD, P], bf16, kind="ExternalInput").ap()
    bup_in = nc.dram_tensor("bup", [P, FT], f32, kind="ExternalInput").ap()
    wdn_in = nc.dram_tensor("wdn", [KD, P, FT, P], bf16, kind="ExternalInput").ap()
    bdn_in = nc.dram_tensor("bdn", [P, KD], f32, kind="ExternalInput").ap()
    cosT_in = nc.dram_tensor("cosT", [P, S], bf16, kind="ExternalInput").ap()
    sinTs_in = nc.dram_tensor("sinTs", [P, S], bf16, kind="ExternalInput").ap()
    rotmT_in = nc.dram_tensor("rotmT", [P, P], bf16, kind="ExternalInput").ap()
    ones_in = nc.dram_tensor("ones", [P, P], f32r, kind="ExternalInput").ap()
    mask_in = nc.dram_tensor("mask", [P, 512], f32, kind="ExternalInput").ap()
    outT = nc.dram_tensor("outT", [D, ROWS], f32, kind="ExternalOutput").ap()

    with tile.TileContext(nc) as tc:
        _emit(tc, nc, xbT_in, xrT_in, wqk_in, bqk_in, wv_in, bv_in, wo_in,
              wup_in, bup_in, wdn_in, bdn_in, cosT_in, sinTs_in, rotmT_in,
              ones_in, mask_in, outT, single_core)

    nc.compile()
    return nc


def _emit(tc, nc, xbT_in, xrT_in, wqk_in, bqk_in, wv_in, bv_in, wo_in,
          wup_in, bup_in, wdn_in, bdn_in, cosT_in, sinTs_in, rotmT_in,
          ones_in, mask_in, outT, single_core=False):
    from contextlib import ExitStack

    top = ExitStack()
    with top:
        consts = top.enter_context(tc.tile_pool(name="consts", bufs=1))
        # only what the first chunk's stats/matmuls need is DMA'd up front;
        # bulky constants are emitted at first use so they don't delay the
        # initial x chunk in the DMA queues.
        ones = consts.tile([P, P], f32r, tag="ones")
        nc.sync.dma_start(ones[:], ones_in)
        ones_bf = consts.tile([P, 1], bf16, tag="ones_bf")
        nc.vector.memset(ones_bf[:], 1.0)
        wmask_sb = consts.tile([P, 512], f32, tag="mask")
        cosT = consts.tile([P, S], bf16, tag="cosT")
        sinTs = consts.tile([P, S], bf16, tag="sinTs")
        rotmT = consts.tile([P, P], bf16, tag="rotmT")
        nc.sync.dma_start(rotmT[:], rotmT_in)
        bqk_sb = consts.tile([P, MQK], f32, tag="bqk")
        nc.sync.dma_start(bqk_sb[:], bqk_in)
        bv_sb = consts.tile([P, NH * P], f32, tag="bv")
        bup_sb = consts.tile([P, FT], f32, tag="bup")
        bdn_sb = consts.tile([P, KD], f32, tag="bdn")
        eps_sb = consts.tile([P, 1], f32, tag="eps")
        nc.vector.memset(eps_sb[:], EPS)
        scr_sb = consts.tile([P, 1], f32, tag="scr")
        nc.vector.memset(scr_sb[:], 0.0)
        sqr2 = consts.tile([1, ROWS], f32, tag="sqr2")
        rr2 = consts.tile([1, ROWS], f32, tag="rr2")

        # DRAM staging for the two AllToAll exchanges (bf16 payload).
        # Split s carries heads [2s, 2s+1]; chunk j holds those heads' dims
        # (256) x core j's 256 owned rows of THIS core's batch.
        dram = top.enter_context(tc.tile_pool(name="a2a", bufs=1, space="DRAM"))
        a2a_in = [dram.tile([N_CORES, HSP * P, RH2], bf16, tag=f"a2a_in{s}",
                            name=f"a2a_in{s}") for s in range(NSPLIT)]
        a2a_out = [dram.tile([N_CORES, HSP * P, RH2], bf16, tag=f"a2a_out{s}",
                             name=f"a2a_out{s}") for s in range(NSPLIT)]

        # x1T (residual accumulator) and the norm2-stats bank outlive the
        # attention pools, so allocate them first (pool scopes are LIFO).
        x1_stack = ExitStack()
        x1p = x1_stack.enter_context(tc.tile_pool(name="x1T", bufs=1))
        x1T = x1p.tile([P, KD * ROWS], f32, tag="x1T")
        # ============ Phase A: norm1 + QK (transposed) + V (natural) ========
        qkv_stack = ExitStack()
        qkvp = qkv_stack.enter_context(tc.tile_pool(name="qkT", bufs=1))
        qkT = qkvp.tile([P, MQK * S], bf16, tag="qkT")
        vnatp = qkv_stack.enter_context(tc.tile_pool(name="vnat", bufs=1))
        # natural-layout V for all 4 heads: block kt is rows [128k,128k+128),
        # head h at columns kt*512 + h*128
        vnat = vnatp.tile([P, QB * NH * P], bf16, tag="vnat")
        wvp = qkv_stack.enter_context(tc.tile_pool(name="wv", bufs=1))
        wv_sb = wvp.tile([P, KD * NH * P], bf16, tag="wv")

        with tc.tile_pool(name="phA_sb", bufs=2) as pa, \
             tc.tile_pool(name="phA_sq", bufs=16) as sqp, \
             tc.tile_pool(name="phA_w", bufs=4) as wp, \
             tc.tile_pool(name="phA_ps", bufs=3, space="PSUM") as pps, \
             tc.tile_pool(name="phA_vps", bufs=2, space="PSUM") as vps, \
             tc.tile_pool(name="phA_st", bufs=2, space="PSUM") as stps, \
             tc.tile_pool(name="phA_rt", bufs=3) as rtp, \
             tc.tile_pool(name="phA_rps", bufs=1, space="PSUM") as rops, \
             tc.tile_pool(name="phA_r", bufs=2) as rp:
            NRH = 512  # rows per chunk
            NCH = S // NRH
            xn1_t = {}

            def load_chunk(c, kd0=0, kd1=KD):
                if kd0 == 0:
                    xn1_t[c] = pa.tile([P, KD * NRH], bf16, tag="xn1",
                                       name=f"xn1_{c}")
                t = xn1_t[c]
                for kd in range(kd0, kd1):
                    nc.sync.dma_start(
                        t[:, kd * NRH:(kd + 1) * NRH],
                        xbT_in[kd * P:(kd + 1) * P, c * NRH:(c + 1) * NRH])

            load_chunk(0)

            def norm_chunk(c):
                # row stats: ssq[r] = sum_d x[d,r]^2 (PE ones-matmul trick),
                # then normalize xn1 in place
                xn1 = xn1_t[c]
                st = stps.tile([P, NRH], f32, tag="st", name=f"st_{c}")
                ssq = st[0:1, :]
                for kd in range(KD):
                    sq = sqp.tile([P, NRH], f32, tag="sq")
                    nc.scalar.activation(sq[:].bitcast(f32r),
                                         xn1[:, kd * NRH:(kd + 1) * NRH],
                                         mybir.ActivationFunctionType.Square)
                    nc.tensor.matmul(
                        ssq, ones[:, 0:1], sq[:].bitcast(f32r),
                        start=(kd == 0), stop=(kd == KD - 1))
                sqr = rp.tile([1, NRH], f32, tag="sqr")
                nc.scalar.activation(sqr[:], ssq,
                                     mybir.ActivationFunctionType.Sqrt,
                                     bias=eps_sb[0:1, :], scale=1.0 / D)
                rr = rp.tile([1, NRH], f32, tag="rr")
                with nc.allow_low_precision(reason="f32r rounding for PE broadcast"):
                    nc.vector.reciprocal(rr[:].bitcast(f32r), sqr[:])
                rb = st
                nc.tensor.matmul(rb[:], ones[0:1, :],
                                 rr[:].bitcast(f32r), start=True, stop=True)
                for kd in range(KD):
                    nc.vector.tensor_mul(xn1[:, kd * NRH:(kd + 1) * NRH],
                                         xn1[:, kd * NRH:(kd + 1) * NRH],
                                         rb[:])

            norm_chunk(0)
            for rh in range(NCH):
                xn1 = xn1_t.pop(rh)
                def v_block(sb):
                    kt = rh * (NRH // P) + sb
                    vacc = vps.tile([P, NH * P], f32, tag="vacc")
                    for kd in range(KD):
                        nc.tensor.matmul(
                            vacc[:],
                            xn1[:, kd * NRH + sb * P: kd * NRH + (sb + 1) * P],
                            wv_sb[:, kd * NH * P:(kd + 1) * NH * P],
                            start=(kd == 0), stop=(kd == KD - 1))
                    nc.vector.tensor_add(
                        vnat[:, kt * NH * P:(kt + 1) * NH * P], vacc[:], bv_sb[:])

                # QK matmuls: head-major m order (q_h = mt h, k_h = mt 4+h)
                # followed by in-place RoPE on this chunk's rows; V-blocks
                # interleave so the DVE work stays spread out.
                for mj, mt in enumerate((0, 4, 1, 5, 2, 6, 3, 7)):
                    wsb = wp.tile([P, KD * P], bf16, tag="wqk")
                    nc.sync.dma_start(
                        wsb[:], wqk_in[mt].rearrange("p k m -> p (k m)"))
                    if rh == 0 and mt == 0:
                        nc.sync.dma_start(cosT[:], cosT_in)
                        nc.sync.dma_start(sinTs[:], sinTs_in)
                    if rh + 1 < NCH:
                        if mt == 4:
                            load_chunk(rh + 1, 0, 5)
                        elif mt == 1:
                            load_chunk(rh + 1, 5, 10)
                        elif mt == 5:
                            load_chunk(rh + 1, 10, KD)
                    acc = pps.tile([P, NRH], f32, tag="qkacc")
                    for kd in range(KD):
                        nc.tensor.matmul(
                            acc[:],
                            wsb[:, kd * P:(kd + 1) * P],
                            xn1[:, kd * NRH:(kd + 1) * NRH],
                            start=(kd == 0), stop=(kd == KD - 1))
                    qk_sl = qkT[:, mt * S + rh * NRH: mt * S + rh * NRH + NRH]
                    # bias-add + cast on DVE (keeps ACT free for the next
                    # chunk's stats squares)
                    nc.vector.tensor_scalar_add(qk_sl, acc[:],
                                                bqk_sb[:, mt:mt + 1])
                    cs_sl = slice(rh * NRH, (rh + 1) * NRH)
                    rt = rops.tile([P, NRH], f32, tag="ropt")
                    nc.tensor.matmul(rt[:], rotmT[:], qk_sl,
                                     start=True, stop=True)
                    rtmp = rtp.tile([P, NRH], bf16, tag="rtmp")
                    nc.vector.tensor_mul(rtmp[:], rt[:], sinTs[:, cs_sl])
                    nc.vector.tensor_mul(qk_sl, qk_sl, cosT[:, cs_sl])
                    nc.vector.tensor_add(qk_sl, qk_sl, rtmp[:])
                if rh == 0:
                    nc.sync.dma_start(bv_sb[:], bv_in)
                    for kd in range(KD):
                        nc.sync.dma_start(
                            wv_sb[:, kd * NH * P:(kd + 1) * NH * P],
                            wv_in[:, kd, :])
                if rh + 1 < NCH:
                    norm_chunk(rh + 1)
                for sb in range(NRH // P):
                    v_block(sb)

        # ====== Phase B + C: attention, split AllToAll, partial O-proj ======
        nc.sync.dma_start(wmask_sb[:], mask_in)
        # preload residual rows (+b_o) straight into x1T
        for kd in range(KD):
            nc.sync.dma_start(x1T[:, kd * ROWS:(kd + 1) * ROWS],
                              xrT_in[kd * P:(kd + 1) * P, :])

        with tc.tile_pool(name="prT", bufs=5) as prtp, \
             tc.tile_pool(name="lsum", bufs=4) as lp, \
             tc.tile_pool(name="rbc", bufs=2) as rbcp, \
             tc.tile_pool(name="ocp", bufs=2) as ocpp, \
             tc.tile_pool(name="oT", bufs=2) as otp, \
             tc.tile_pool(name="phC_om", bufs=2) as omp, \
             tc.tile_pool(name="phC_w", bufs=32) as wop, \
             tc.tile_pool(name="phC_sq", bufs=3) as sqp2, \
             tc.tile_pool(name="sc_ps", bufs=2, space="PSUM") as scps, \
             tc.tile_pool(name="ov_ps", bufs=1, space="PSUM") as ovps, \
             tc.tile_pool(name="phC_ps", bufs=2, space="PSUM") as cps, \
             tc.tile_pool(name="st_ps", bufs=1, space="PSUM") as stp2:

            def attention_head(h):
                rq = qkT[:, h * S:(h + 1) * S]
                rk = qkT[:, (NH + h) * S:(NH + h + 1) * S]
                oTh = otp.tile([P, S], bf16, tag="oTh")
                # q processed in 512-wide chunks; scores computed TRANSPOSED
                # (s.T[S_k, q]) so exp output is already in PV layout.
                # Software pipelining: each kt's lps/PV matmuls are deferred
                # two score-matmuls so the PE never waits on ACT exp, and
                # each qc's 1/l normalization is deferred into the next qc
                # so the PE never waits on the DVE reciprocal.
                pend = [None]

                def flush():
                    if pend[0] is None:
                        return
                    ocopy_p, lr_p, rinv_p, dst = pend[0]
                    pend[0] = None
                    nc.tensor.matmul(lr_p[:], ones[0:1, :],
                                     rinv_p[:].bitcast(f32r),
                                     start=True, stop=True)
                    rbs = rbcp.tile([P, 512], f32, tag="rbs")
                    nc.vector.tensor_copy(rbs[:], lr_p[:])
                    nc.vector.tensor_mul(dst, ocopy_p[:], rbs[:])

                for qc in (3, 2, 1, 0):
                    opsum = ovps.tile([P, 512], f32, tag="opv")
                    lr = stp2.tile([P, 512], f32, tag="lr")
                    lps = lr[0:1, :]
                    nkt = 4 * qc + 4

                    def lps_pv(prT_sl, kt, q0, w, nkt=nkt, lr=lr,
                               opsum=opsum, h=h):
                        # partial-width accumulates: causal diagonal blocks
                        # only cover q columns >= their own k rows
                        nc.tensor.matmul(
                            lr[0:1, q0:512], ones_bf[:], prT_sl,
                            start=(kt == 0), stop=(kt == nkt - 1),
                            skip_group_check=True)
                        nc.tensor.matmul(
                            opsum[:, q0:512],
                            vnat[:, kt * NH * P + h * P: kt * NH * P + (h + 1) * P],
                            prT_sl,
                            start=(kt == 0), stop=(kt == nkt - 1),
                            skip_group_check=True)

                    todo = []
                    emitted = [0]

                    def drain_todo(upto):
                        while emitted[0] < upto:
                            lps_pv(*todo[emitted[0]])
                            emitted[0] += 1

                    pair = [None]
                    for kt in range(nkt):
                        lb = kt - 4 * qc
                        q0 = max(lb, 0) * P
                        w = 512 - q0
                        if lb < 0:
                            # full-width block: pack two per PSUM pair-tile,
                            # one exp call per pair (ACT dispatch is pricey)
                            if pair[0] is None:
                                pt = scps.tile([P, 1024], f32, tag="scc")
                                pair[0] = (pt, [])
                            pt, members = pair[0]
                            half = len(members)
                            nc.tensor.matmul(
                                pt[:, half * 512:(half + 1) * 512],
                                rk[:, kt * P:(kt + 1) * P],
                                rq[:, qc * 512:(qc + 1) * 512],
                                start=True, stop=True)
                            members.append(kt)
                            if kt == 0:
                                flush()
                            if len(members) == 2:
                                prT = prtp.tile([P, 1024], bf16, tag="prT")
                                nc.scalar.activation(
                                    prT[:], pt[:],
                                    mybir.ActivationFunctionType.Exp,
                                    scale=SCALE)
                                todo.append((prT[:, 0:512], members[0], 0, 512))
                                todo.append((prT[:, 512:1024], members[1], 0, 512))
                                pair[0] = None
                        else:
                            scc = scps.tile([P, 1024], f32, tag="scc")
                            nc.tensor.matmul(
                                scc[:, 0:w],
                                rk[:, kt * P:(kt + 1) * P],
                                rq[:, qc * 512 + q0:(qc + 1) * 512],
                                start=True, stop=True)
                            if kt == 0:
                                flush()
                            # triangular mask on the block-diagonal 128 cols
                            nc.vector.tensor_add(
                                scc[:, 0:P], scc[:, 0:P],
                                wmask_sb[:, 384:512])
                            prT = prtp.tile([P, 1024], bf16, tag="prT")
                            nc.scalar.activation(
                                prT[:, 0:w], scc[:, 0:w],
                                mybir.ActivationFunctionType.Exp, scale=SCALE)
                            todo.append((prT[:, 0:w], kt, q0, w))
                        drain_todo(len(todo) - (6 if nkt > 4 else 2))
                    drain_todo(nkt)
                    rinv = lp.tile([1, 512], f32, tag="rinv")
                    with nc.allow_low_precision(reason="f32r rounding for PE bcast"):
                        nc.vector.reciprocal(rinv[:].bitcast(f32r), lps)
                    # evacuate the PV sum to SBUF so the PSUM bank frees now
                    ocopy = ocpp.tile([P, 512], f32, tag="ocopy")
                    nc.vector.tensor_copy(ocopy[:], opsum[:])
                    pend[0] = (ocopy, lr, rinv,
                               oTh[:, qc * 512:(qc + 1) * 512])
                flush()
                # stage this head's output for its a2a split (single DMA:
                # peer dim j is just the 256-col blocking of oTh)
                s, hh = divmod(h, HSP)
                nc.sync.dma_start(
                    a2a_in[s][:, hh * P:(hh + 1) * P, :]
                    .rearrange("j p r -> p j r"),
                    oTh[:].rearrange("p (j r) -> p j r", r=RH2))

            def exchange(s):
                if single_core:
                    # timing-only stand-in for the collective: move the same
                    # bytes DRAM->DRAM locally
                    nc.sync.dma_start(
                        a2a_out[s][:].rearrange("a r c -> (a r) c"),
                        a2a_in[s][:].rearrange("a r c -> (a r) c"))
                else:
                    nc.gpsimd.collective_compute(
                        "AllToAll", mybir.AluOpType.bypass,
                        replica_groups=GROUPS,
                        ins=[a2a_in[s][:].opt()],
                        outs=[a2a_out[s][:].opt()],
                    )

            NK = KD // NSPLIT
            wo_tiles = {}

            def load_wo(s):
                for mt in range(KD):
                    wsb = wop.tile([P, NK * P], bf16, tag="wo",
                                   name=f"wo_{s}_{mt}")
                    nc.sync.dma_start(
                        wsb[:], wo_in[mt][:, s * NK:(s + 1) * NK, :]
                        .rearrange("p k m -> p (k m)"))
                    wo_tiles[(s, mt)] = wsb

            def oproj_partial(s):
                # split s delivers kds {4g + 2s, 4g + 2s + 1 : g in 0..3}
                kds = [4 * g + HSP * s + i for g in range(4) for i in range(HSP)]
                om = omp.tile([P, len(kds) * ROWS], bf16, tag="om")
                for ci, kd in enumerate(kds):
                    g, i = kd // 4, kd % 4 - HSP * s
                    nc.sync.dma_start(
                        om[:, ci * ROWS:(ci + 1) * ROWS]
                        .rearrange("p (b r) -> p b r", r=RH2),
                        a2a_out[s][g::4, i * P:(i + 1) * P, :]
                        .rearrange("b p r -> p b r"))
                nk = len(kds)
                for mt in range(KD):
                    wsb = wo_tiles.pop((s, mt))
                    acc = cps.tile([P, ROWS], f32, tag="oacc")
                    for ci in range(nk):
                        nc.tensor.matmul(
                            acc[:], wsb[:, ci * P:(ci + 1) * P],
                            om[:, ci * ROWS:(ci + 1) * ROWS],
                            start=(ci == 0), stop=(ci == nk - 1))
                    nc.vector.tensor_add(x1T[:, mt * ROWS:(mt + 1) * ROWS],
                                         x1T[:, mt * ROWS:(mt + 1) * ROWS],
                                         acc[:])
                    if s == NSPLIT - 1:
                        # x1T[mt] final: fold norm2 stats in right away
                        if mt == 0:
                            st2_t[0] = stp2.tile([P, 512], f32, tag="lr",
                                                 name="st2")
                        sq = sqp2.tile([P, ROWS], f32, tag="sq2")
                        nc.scalar.activation(
                            sq[:].bitcast(f32r), x1T[:, mt * ROWS:(mt + 1) * ROWS],
                            mybir.ActivationFunctionType.Square)
                        nc.tensor.matmul(st2_t[0][0:1, :], ones[:, 0:1],
                                         sq[:].bitcast(f32r),
                                         start=(mt == 0), stop=(mt == KD - 1))

            # pipeline: each split's exchange flies while the next head's
            # attention keeps the PE busy; its O-proj partial lands after.
            for h in range(NH):
                attention_head(h)
                if (h + 1) % HSP == 0:
                    load_wo((h + 1) // HSP - 1)
                    if (h + 1) // HSP - 2 >= 0:
                        oproj_partial((h + 1) // HSP - 2)
                    exchange((h + 1) // HSP - 1)
            # preload the sqrt ACT table while the last O-proj runs
            nc.scalar.activation(scr_sb[:], scr_sb[:],
                                 mybir.ActivationFunctionType.Sqrt)
            st2_t = [None]
            oproj_partial(NSPLIT - 1)
            # norm2 scale while the stats bank is still allocated
            nc.scalar.activation(sqr2[:], st2_t[0][0:1, :],
                                 mybir.ActivationFunctionType.Sqrt,
                                 bias=eps_sb[0:1, :], scale=1.0 / D)
            with nc.allow_low_precision(reason="f32r rounding for PE broadcast"):
                nc.vector.reciprocal(rr2[:].bitcast(f32r), sqr2[:])
        qkv_stack.close()

        # ================= Phase D..F: norm2 + MLP ==========================
        mlp_stack = ExitStack()
        xn2p = mlp_stack.enter_context(tc.tile_pool(name="xn2", bufs=1))
        fnp = mlp_stack.enter_context(tc.tile_pool(name="fnT", bufs=1))
        xn2 = xn2p.tile([P, KD * ROWS], bf16, tag="xn2")
        fnT = fnp.tile([P, FT * ROWS], bf16, tag="fnT")

        nc.sync.dma_start(bup_sb[:], bup_in)
        nc.sync.dma_start(bdn_sb[:], bdn_in)
        with tc.tile_pool(name="phD_rb", bufs=1, space="PSUM") as rbps, \
             tc.tile_pool(name="phE_w", bufs=4) as wup_p, \
             tc.tile_pool(name="phF_w", bufs=2) as wdn_p, \
             tc.tile_pool(name="phF_out", bufs=2) as outp, \
             tc.tile_pool(name="phE_ps", bufs=4, space="PSUM") as eps_ps, \
             tc.tile_pool(name="phF_ps", bufs=3, space="PSUM") as fps:
            rb2 = rbps.tile([P, ROWS], f32, tag="rb2")
            nc.tensor.matmul(rb2[:], ones[0:1, :],
                             rr2[:].bitcast(f32r), start=True, stop=True)
            for kd in range(KD):
                nc.vector.tensor_mul(xn2[:, kd * ROWS:(kd + 1) * ROWS],
                                     x1T[:, kd * ROWS:(kd + 1) * ROWS], rb2[:])

            for mt in range(FT):
                wsb = wup_p.tile([P, KD * P], bf16, tag="wup")
                nc.sync.dma_start(wsb[:], wup_in[mt].rearrange("p k m -> p (k m)"))
                acc = eps_ps.tile([P, ROWS], f32, tag="upacc")
                for kd in range(KD):
                    nc.tensor.matmul(
                        acc[:], wsb[:, kd * P:(kd + 1) * P],
                        xn2[:, kd * ROWS:(kd + 1) * ROWS],
                        start=(kd == 0), stop=(kd == KD - 1))
                # fn = silu(up + b_up), cast to bf16
                nc.scalar.activation(fnT[:, mt * ROWS:(mt + 1) * ROWS], acc[:],
                                     mybir.ActivationFunctionType.Silu,
                                     bias=bup_sb[:, mt:mt + 1])

            for mt in range(KD):
                wsb = wdn_p.tile([P, FT * P], bf16, tag="wdn")
                nc.sync.dma_start(wsb[:], wdn_in[mt].rearrange("p k m -> p (k m)"))
                acc = fps.tile([P, ROWS], f32, tag="dnacc")
                for kd in range(FT):
                    nc.tensor.matmul(
                        acc[:], wsb[:, kd * P:(kd + 1) * P],
                        fnT[:, kd * ROWS:(kd + 1) * ROWS],
                        start=(kd == 0), stop=(kd == FT - 1))
                out_sb = outp.tile([P, ROWS], f32, tag="out_sb")
                for hh in range(2):
                    cs = slice(hh * (ROWS // 2), (hh + 1) * (ROWS // 2))
                    nc.vector.scalar_tensor_tensor(
                        out_sb[:, cs], acc[:, cs], bdn_sb[:, mt:mt + 1],
                        x1T[:, mt * ROWS + hh * (ROWS // 2):
                            mt * ROWS + (hh + 1) * (ROWS // 2)],
                        op0=mybir.AluOpType.add, op1=mybir.AluOpType.add)
                    nc.sync.dma_start(outT[mt * P:(mt + 1) * P, cs],
                                      out_sb[:, cs])
        mlp_stack.close()
        x1_stack.close()


def host_prepare(inputs):
    """Fold LoRA/norm-weights/biases and build the 8 per-core input maps."""
    gi = {k: np.asarray(v, dtype=np.float32) if np.asarray(v).dtype != np.float32
          else np.asarray(v) for k, v in inputs.items()}

    def fold(nm):
        return gi['w_' + nm] + gi['w_' + nm + '_lora_a'] @ gi['w_' + nm + '_lora_b']

    nw1 = gi['norm_weight_1'][:, None]
    nw2 = gi['norm_weight_2'][:, None]
    w_q = (nw1 * fold('q')).astype(np.float32)
    w_k = (nw1 * fold('k')).astype(np.float32)
    w_v = (nw1 * fold('v')).astype(np.float32)
    w_o = fold('o').astype(np.float32)
    w_up = (nw2 * fold('up')).astype(np.float32)
    w_dn = fold('down').astype(np.float32)

    # pre-tiled weight layouts [mt, p, kd, m]
    kd_order = [4 * g + HSP * sp + i
                for sp in range(NSPLIT) for g in range(4) for i in range(HSP)]
    wo_t = np.ascontiguousarray(
        w_o.reshape(KD, P, KD, P).transpose(2, 1, 0, 3)[:, :, kd_order]).astype(
            ml_dtypes.bfloat16)
    wup_t = np.ascontiguousarray(
        w_up.reshape(KD, P, FT, P).transpose(2, 1, 0, 3)).astype(ml_dtypes.bfloat16)
    wdn_t = np.ascontiguousarray(
        w_dn.reshape(FT, P, KD, P).transpose(2, 1, 0, 3)).astype(ml_dtypes.bfloat16)
    bup_t = np.ascontiguousarray(gi['b_up'].reshape(FT, P).T)
    bdn_t = np.ascontiguousarray(gi['b_down'].reshape(KD, P).T)

    cosT = np.ascontiguousarray(gi['cos'].T).astype(ml_dtypes.bfloat16)
    sinTs = np.ascontiguousarray(gi['sin'].T).astype(ml_dtypes.bfloat16)
    # rot(x).T = R @ x.T with R[d, d+64] = -1 (d<64), R[d, d-64] = +1;
    # matmul computes lhsT.T @ rhs, so pass R.T.
    Rm = np.zeros((P, P), dtype=np.float32)
    hh = HD // 2
    Rm[np.arange(hh), np.arange(hh) + hh] = -1.0
    Rm[np.arange(hh) + hh, np.arange(hh)] = 1.0
    rotmT = np.ascontiguousarray(Rm.T).astype(ml_dtypes.bfloat16)
    maskT = np.maximum(gi['attention_mask'][0, 0, :P, :P], -2000.0).T
    wmask = np.full((P, 512), -2000.0, dtype=np.float32)
    wmask[:, 384:512] = maskT
    mask128 = np.ascontiguousarray(wmask)

    x = gi['x']
    b_o = gi['b_o']
    in_maps = []
    for i in range(N_CORES):
        b, g = divmod(i, 4)
        hs = slice(512 * g, 512 * (g + 1))
        wqk = np.concatenate([w_q[:, hs], w_k[:, hs]], axis=1)
        wqk_t = np.ascontiguousarray(
            wqk.reshape(KD, P, MQK, P).transpose(2, 1, 0, 3)).astype(ml_dtypes.bfloat16)
        bqk = np.concatenate([gi['b_q'][hs], gi['b_k'][hs]])
        bqk_t = np.ascontiguousarray(bqk.reshape(MQK, P).T)
        # V natural-layout weights: [p, kd, m] so the kd-th moving slice is
        # w_v rows [128kd, 128kd+128) x this core's 512 head-dims
        wv_t = np.ascontiguousarray(
            w_v[:, hs].reshape(KD, P, NH * P).transpose(1, 0, 2)).astype(
                ml_dtypes.bfloat16)
        bv_t = np.broadcast_to(gi['b_v'][hs], (P, NH * P)).copy()
        xbT = np.ascontiguousarray(x[b].T).astype(ml_dtypes.bfloat16)
        # this core owns rows [256i, 256(i+1)) of BOTH batches
        xrows = np.concatenate(
            [x[0, RH2 * i:RH2 * (i + 1)], x[1, RH2 * i:RH2 * (i + 1)]], axis=0)
        xrT = np.ascontiguousarray(xrows.T + b_o[:, None])
        in_maps.append({
            "xbT": xbT, "xrT": xrT,
            "wqk": wqk_t, "bqk": bqk_t, "wv": wv_t, "bv": bv_t,
            "wo": wo_t, "wup": wup_t, "bup": bup_t,
            "wdn": wdn_t, "bdn": bdn_t,
            "cosT": cosT, "sinTs": sinTs, "rotmT": rotmT,
            "ones": np.ones((P, P), dtype=np.float32), "mask": mask128,
        })
    return in_maps


def assemble(results):
    out = np.empty((B, S, D), dtype=np.float32)
    for i in range(N_CORES):
        oT = results[i]["outT"]
        out[0, RH2 * i:RH2 * (i + 1), :] = oT[:, 0:RH2].T
        out[1, RH2 * i:RH2 * (i + 1), :] = oT[:, RH2:ROWS].T
    return out


_NC_CACHE = {}


def get_nc():
    if "nc" not in _NC_CACHE:
        _NC_CACHE["nc"] = build_program()
    return _NC_CACHE["nc"]


def kernel(**inputs):
    nc = get_nc()
    in_maps = host_prepare(inputs)
    res = run_bass_kernel_spmd(nc, in_maps, list(range(N_CORES)))
    return assemble(res.results)
